# revision 5
# baseline (speedup 1.0000x reference)
"""Trainium2 Bass kernel for nn_ExteriorDerivative (d of a 2-form via central FD).

Math: the reference's central finite difference collapses analytically:
  (x +/- eps e_d) @ W1 = z +/- eps*W1[d]  with z = x @ W1, and
  sin(z+a) - sin(z-a) = 2 cos(z) sin(a), so
  fd[d] = cos(z) @ (diag(sin(eps*W1[d])/eps) @ W2)
and the whole gather/sign/scatter pipeline folds into one (32, 35) matrix G:
  out = cos(x @ W1) @ G = g1 + (sin(z/2)^2) @ (-2 G),   g1 = G.sum(0)
using cos(z) = 1 - 2 sin^2(z/2)  (|z/2| < pi here, no range reduction).

Device pipeline per core (pure batch-parallel across 8 cores; 32768
samples/core packed 4 subgroups x 8192 columns so every elementwise tile
uses all 128 partitions; engine time in this regime is per *column*):
  mm1:  z = blockdiag(W1 x4)^T @ xt            [128, 1024] PSUM f32 (f16 mm)
  s   = Sin(0.5 z)                             ACT, PSUM->SBUF f16
  q   = s*s                                    DVE scalar_tensor_tensor (4x)
                                               or Pool tensor_tensor (sched)
  mm2A: T1 = GA^T @ q                          [128, 512] PSUM
        GA [128,128] = blockdiag(-2G x3) plus rows 96:128 -> outputs 0:23
        of subgroup 3, so T1 rows are 3 full samples + 23/35 of a 4th.
  copy: och = T1 + g1 (per-partition f16 bias) DVE tensor_scalar / ACT Ident
        (GPSIMD cannot access PSUM, so copies split DVE/ACT only)
  mm2B: leftover 12 outputs of subgroup 3 via ldweights-stationary matmuls:
        T2[128 samp, 12] += q[96:128, blk]^T @ (-2G)[:, 23:35], with a
        rank-1 ones x g1[23:35] prefill matmul providing the bias.
  copy: o2 = T2 (pure DVE copy), DMA out       ot [128, 8960] f16
DMA descriptor-gen on SP/HWDGE (first x chunk via Pool SWDGE to overlap
startup latency); input x shipped f16; weights in one f16 blob.
"""
import numpy as np
from itertools import combinations

DIM = 7
EPS = 1e-4
NCORES = 8
B = 262144
B_CORE = B // NCORES          # 32768
SUB = 4                       # subgroups stacked on partitions
COLS = B_CORE // SUB          # 8192 columns per core
K_IN = SUB * DIM              # 28 input partitions
GROUP = 1024                  # columns per z tile (2 psum banks)
PAIR = 2 * GROUP              # columns per s/q tile and per load/store
NPAIR = COLS // PAIR          # 4
T2_BLK = 128                  # samples per mm2B matmul (stationary free dim)
T2_PER_TILE = 32              # mm2B blocks per psum tile (32*12=384 <= 512)
T2_W = T2_PER_TILE * 12       # 384
NT2 = COLS // T2_BLK          # 64 blocks -> 2 tiles of 32
OUT_W = COLS + NT2 * 12       # 8960 output columns

# blob column layout (f16, [128, BW])
BC_W1 = 0                     # [0:28, 0:128]     blockdiag W1 x4
BC_GA = 128                   # [0:128, 128:256]  GA
BC_G2 = 256                   # [96:128, 256:268] (-2G)[:,23:35], partitions 96..127
BC_ONES = 268                 # [0:1, 268:396]    ones row (prefill lhsT)
BC_G1T2 = 396                 # [0:1, 396:780]    g1[23:35] tiled x32 (prefill rhs)
BC_G1S = 780                  # [0:128, 780:781]  g1s128 column (copy bias)
BW = 781

import os as _os
# T1-copy engine schedule (16 chunks of 512): 'd' DVE / 'a' ACT
COPY_SCHED = _os.environ.get("K_COPYSCHED", "dddadddadddaddda")
# square engine per 1024-group (8 groups): 'd' DVE STT / 'p' Pool TT
SQ_SCHED = _os.environ.get("K_SQSCHED", "dppddppd")

# ---- static exterior-derivative index maps (mirrors reference.py) ----
_IDX3 = list(combinations(range(DIM), 3))
_POS2 = {t: i for i, t in enumerate(combinations(range(DIM), 2))}
_D2 = []
for _out, (i, j, k) in enumerate(_IDX3):
    for _p, (a, b, c) in enumerate([(i, j, k), (j, i, k), (k, i, j)]):
        bc = tuple(sorted((b, c)))
        s = (-1) ** _p * (1 if (b, c) == bc else -1)
        _D2.append((_out, _POS2[bc], a, s))


def _build_G(W1: np.ndarray, W2: np.ndarray) -> np.ndarray:
    """G[j, o] = sum_t SIGNS[t] * sin(EPS*W1[DCOORD[t], j])/EPS * W2[j, IN_POS[t]]  (fp64)."""
    W1d = W1.astype(np.float64)
    W2d = W2.astype(np.float64)
    G = np.zeros((32, 35), dtype=np.float64)
    for out_pos, in_pos, dcoord, sign in _D2:
        G[:, out_pos] += sign * (np.sin(EPS * W1d[dcoord, :]) / EPS) * W2d[:, in_pos]
    return G


_PROG = None


def _get_prog():
    global _PROG
    if _PROG is not None:
        return _PROG
    import concourse.bacc as bacc
    import concourse.bass as bass
    import concourse.tile as tile
    import concourse.mybir as mybir
    from concourse.alu_op_type import AluOpType as Alu

    F32 = mybir.dt.float32
    F16 = mybir.dt.float16
    Sin = mybir.ActivationFunctionType.Sin
    Ident = mybir.ActivationFunctionType.Identity

    nc = bacc.Bacc("TRN2", target_bir_lowering=False, debug=False, num_devices=NCORES)
    xt = nc.dram_tensor("xt", [K_IN, COLS], F16, kind="ExternalInput")
    blob = nc.dram_tensor("blob", [128, BW], F16, kind="ExternalInput")
    g1f32 = nc.dram_tensor("g1f32", [128, 1], F32, kind="ExternalInput")
    ot = nc.dram_tensor("ot", [128, OUT_W], F16, kind="ExternalOutput")

    with tile.TileContext(nc) as tc:
        with (
            tc.tile_pool(name="singles", bufs=1) as singles,
            tc.tile_pool(name="xin", bufs=2) as xpool,
            tc.tile_pool(name="och", bufs=2) as opool,
            tc.tile_pool(name="o2ch", bufs=2) as o2pool,
            tc.tile_pool(name="ssp", bufs=2) as spool,
            tc.tile_pool(name="qqp", bufs=2) as qpool,
            tc.tile_pool(name="zps", bufs=2, space=bass.MemorySpace.PSUM) as zpsum,
            tc.tile_pool(name="t1ps", bufs=2, space=bass.MemorySpace.PSUM) as t1psum,
            tc.tile_pool(name="t2ps", bufs=2, space=bass.MemorySpace.PSUM) as t2psum,
        ):
            bl = singles.tile([128, BW], F16)
            nc.sync.dma_start(bl[:], blob[:])
            g1s = singles.tile([128, 1], F32)
            nc.sync.dma_start(g1s[:], g1f32[:])

            w1_ap = bl[0:K_IN, BC_W1:BC_W1 + 128]
            ga_ap = bl[0:128, BC_GA:BC_GA + 128]
            g2_ap = bl[96:128, BC_G2:BC_G2 + 12]
            ones_ap = bl[0:1, BC_ONES:BC_ONES + 128]
            g1t2_ap = bl[0:1, BC_G1T2:BC_G1T2 + T2_W]

            t2 = None
            chunk = 0          # T1-copy chunk counter (16 total)
            for p in range(NPAIR):
                c0 = p * PAIR
                xin = xpool.tile([K_IN, PAIR], F16, tag="xin")
                # first chunk goes out on Pool's SWDGE so its descriptor gen
                # overlaps the blob load on SP/HWDGE during the startup window
                (nc.gpsimd if p == 0 else nc.sync).dma_start(
                    xin[:], xt[:, c0:c0 + PAIR])
                och = opool.tile([128, PAIR], F16, tag="och")
                ss = spool.tile([128, PAIR], F16, tag="ss")
                qq = qpool.tile([128, PAIR], F16, tag="qq")

                for h in range(2):
                    ho = h * GROUP
                    zp = zpsum.tile([128, GROUP], F32, tag="zp")
                    for s in range(0, GROUP, 512):
                        nc.tensor.matmul(zp[:, s:s + 512], w1_ap,
                                         xin[:, ho + s:ho + s + 512])
                    nc.scalar.activation(ss[:, ho:ho + GROUP], zp[:], Sin,
                                         bias=0.0, scale=0.5)
                    if SQ_SCHED[(2 * p + h) % len(SQ_SCHED)] == 'p':
                        nc.gpsimd.tensor_tensor(qq[:, ho:ho + GROUP],
                                                ss[:, ho:ho + GROUP],
                                                ss[:, ho:ho + GROUP], Alu.mult)
                    else:
                        nc.vector.scalar_tensor_tensor(qq[:, ho:ho + GROUP],
                                                       ss[:, ho:ho + GROUP], 1.0,
                                                       ss[:, ho:ho + GROUP],
                                                       Alu.mult, Alu.mult)

                    # main output stream: 3 samples + 23/35 of a 4th per col
                    for s in range(0, GROUP, 512):
                        t1 = t1psum.tile([128, 512], F32, tag="t1")
                        nc.tensor.matmul(t1[:], ga_ap, qq[:, ho + s:ho + s + 512])
                        dst = och[:, ho + s:ho + s + 512]
                        e = COPY_SCHED[chunk % len(COPY_SCHED)]
                        chunk += 1
                        if e == 'a':
                            nc.scalar.activation(dst, t1[:], Ident,
                                                 bias=g1s[:], scale=1.0)
                        else:
                            nc.vector.tensor_scalar(dst, t1[:], g1s[:], None,
                                                    Alu.add)

                    # leftover 12 outputs of subgroup 3 via q-stationary mms
                    for b in range(GROUP // T2_BLK):
                        tb = (2 * p + h) * (GROUP // T2_BLK) + b
                        w = tb % T2_PER_TILE
                        if w == 0:
                            t2 = t2psum.tile([128, T2_W], F32, tag="t2")
                            nc.tensor.matmul(t2[:], ones_ap, g1t2_ap,
                                             start=True, stop=False)
                        qb = qq[96:128, ho + b * T2_BLK:ho + (b + 1) * T2_BLK]
                        nc.tensor.matmul(t2[:, 12 * w:12 * w + 12], qb, g2_ap,
                                         start=False, stop=(w == T2_PER_TILE - 1),
                                         tile_position=(96, 0))
                        if w == T2_PER_TILE - 1:
                            o2 = o2pool.tile([128, T2_W], F16, tag="o2")
                            nc.vector.tensor_copy(o2[:], t2[:])
                            t2o = COLS + (tb // T2_PER_TILE) * T2_W
                            nc.sync.dma_start(ot[:, t2o:t2o + T2_W], o2[:])

                if p < NPAIR - 1:
                    nc.sync.dma_start(ot[:, c0:c0 + PAIR], och[:])
                else:
                    # split the last store so the tail transfer is short
                    nc.sync.dma_start(ot[:, c0:c0 + PAIR - 512],
                                      och[:, :PAIR - 512])
                    nc.sync.dma_start(ot[:, c0 + PAIR - 512:c0 + PAIR],
                                      och[:, PAIR - 512:])

    nc.compile()
    _PROG = nc
    return nc


def _pack_inputs(x: np.ndarray, W1: np.ndarray, W2: np.ndarray):
    assert x.shape == (B, DIM), x.shape
    assert W1.shape == (DIM, 32), W1.shape
    assert W2.shape == (32, 21), W2.shape
    G = _build_G(W1, W2)                      # fp64 (32, 35)
    Gm2 = (-2.0 * G).astype(np.float16)       # (32, 35)
    g1 = G.sum(axis=0)                        # (35,)

    g1s128 = np.empty(128, dtype=np.float64)
    for h in range(3):
        g1s128[35 * h:35 * h + 35] = g1
    g1s128[105:128] = g1[:23]

    blob = np.zeros((128, BW), dtype=np.float16)
    for gsub in range(SUB):
        blob[7 * gsub:7 * gsub + 7, BC_W1 + 32 * gsub:BC_W1 + 32 * gsub + 32] = \
            W1.astype(np.float16)
    for h in range(3):
        blob[32 * h:32 * h + 32, BC_GA + 35 * h:BC_GA + 35 * h + 35] = Gm2
    blob[96:128, BC_GA + 105:BC_GA + 128] = Gm2[:, :23]
    blob[96:128, BC_G2:BC_G2 + 12] = Gm2[:, 23:35]
    blob[0, BC_ONES:BC_ONES + 128] = 1.0
    blob[0, BC_G1T2:BC_G1T2 + T2_W] = np.tile(g1[23:35], T2_PER_TILE).astype(np.float16)

    # xt[m][7g+f, c] = x[m*B_CORE + g*COLS + c, f]
    xr = np.asarray(x, dtype=np.float16).reshape(NCORES, SUB, COLS, DIM)
    xt = np.ascontiguousarray(xr.transpose(0, 1, 3, 2).reshape(NCORES, K_IN, COLS))
    g1f = np.ascontiguousarray(g1s128[:, None], dtype=np.float32)
    in_maps = [{"xt": xt[m], "blob": blob, "g1f32": g1f} for m in range(NCORES)]
    return in_maps


def _unpack_outputs(results) -> np.ndarray:
    ot = np.stack([r["ot"] for r in results])       # (8, 128, 8960) f16
    A = ot[:, :, :COLS]                             # (8, 128, 8192)
    # subgroups 0..2: rows 35h+o
    a3 = A[:, :105, :].reshape(NCORES, 3, 35, COLS).transpose(0, 1, 3, 2)
    # subgroup 3 outputs 0..22: rows 105..127
    a4 = A[:, 105:128, :].transpose(0, 2, 1)        # (8, 8192, 23)
    # subgroup 3 outputs 23..34: T2 region [128 samples, 12] blocks
    Bp = ot[:, :, COLS:].reshape(NCORES, 128, NT2 // T2_PER_TILE, T2_PER_TILE, 12)
    b4 = Bp.transpose(0, 2, 3, 1, 4).reshape(NCORES, COLS, 12)
    out4 = np.concatenate([a4, b4], axis=2)         # (8, 8192, 35)
    out = np.concatenate([a3.reshape(NCORES, 3 * COLS, 35), out4], axis=1)
    return np.ascontiguousarray(out.reshape(B, 35), dtype=np.float32)


def run(x, W1, W2, **spmd_kwargs):
    """Run the kernel; returns (output, BassKernelResults)."""
    from concourse.bass_utils import run_bass_kernel_spmd
    nc = _get_prog()
    in_maps = _pack_inputs(np.asarray(x, dtype=np.float32),
                           np.asarray(W1, dtype=np.float32),
                           np.asarray(W2, dtype=np.float32))
    res = run_bass_kernel_spmd(nc, in_maps, core_ids=list(range(NCORES)), **spmd_kwargs)
    return _unpack_outputs(res.results), res


def kernel(x, W1, W2):
    out, _ = run(x, W1, W2)
    return out


# revision 17
# speedup vs baseline: 1.1688x; 1.1688x over previous
"""Trainium2 Bass kernel for nn_ExteriorDerivative (d of a 2-form via central FD).

Math: the reference's central finite difference collapses analytically:
  (x +/- eps e_d) @ W1 = z +/- eps*W1[d]  with z = x @ W1, and
  sin(z+a) - sin(z-a) = 2 cos(z) sin(a), so
  fd[d] = cos(z) @ (diag(sin(eps*W1[d])/eps) @ W2)
and the whole gather/sign/scatter pipeline folds into one (32, 35) matrix G:
  out = cos(x @ W1) @ G = g1 + (sin(z/2)^2) @ (-2 G),   g1 = G.sum(0)
using cos(z) = 1 - 2 sin^2(z/2)  (|z/2| < pi here, no range reduction).

Device pipeline per core (pure batch-parallel across 8 cores; 32768
samples/core packed 4 subgroups x 8192 columns so every elementwise tile
uses all 128 partitions; engine time in this regime is per *column*):
  mm1:  z = blockdiag(W1 x4)^T @ xt            [128, 1024] PSUM f32 (f16 mm)
  s   = Sin(0.5 z)                             ACT, PSUM->SBUF f16
  q   = s*s                                    DVE scalar_tensor_tensor (4x)
                                               or Pool tensor_tensor (sched)
  mm2A: T1 = GA^T @ q                          [128, 512] PSUM
        GA [128,128] = blockdiag(-2G x3) plus rows 96:128 -> outputs 0:23
        of subgroup 3, so T1 rows are 3 full samples + 23/35 of a 4th.
  copy: och = T1 + g1 (per-partition f16 bias) DVE tensor_scalar / ACT Ident
        (GPSIMD cannot access PSUM, so copies split DVE/ACT only)
  mm2B: leftover 12 outputs of subgroup 3 via ldweights-stationary matmuls:
        T2[128 samp, 12] += q[96:128, blk]^T @ (-2G)[:, 23:35], with a
        rank-1 ones x g1[23:35] prefill matmul providing the bias.
  copy: o2 = T2 (pure DVE copy), DMA out       ot [128, 8960] f16
DMA descriptor-gen on SP/HWDGE (first x chunk via Pool SWDGE to overlap
startup latency); input x shipped f16; weights in one f16 blob.
"""
import numpy as np
from itertools import combinations

DIM = 7
EPS = 1e-4
NCORES = 8
B = 262144
B_CORE = B // NCORES          # 32768
SUB = 4                       # subgroups stacked on partitions
COLS = B_CORE // SUB          # 8192 columns per core
K_IN = SUB * DIM              # 28 input partitions
GROUP = 1024                  # columns per z tile (2 psum banks)
PAIR = 2 * GROUP              # columns per s/q tile and per load/store
NPAIR = COLS // PAIR          # 4
NGRP = COLS // GROUP          # 8
T2_BLK = 128                  # samples per mm2B matmul (stationary free dim)
T2_PER_TILE = 32              # mm2B blocks per psum tile (32*12=384, 1 bank)
T2_W = T2_PER_TILE * 12       # 384
NT2 = COLS // T2_BLK          # 64 blocks -> 2 tiles of 32
OUT_W = COLS + NT2 * 12       # 8960 output columns

# blob column layout (f16, [128, BW])
BC_W1 = 0                     # [0:28, 0:128]     blockdiag W1 x4
BC_GA = 128                   # [0:128, 128:256]  GA
BC_G2 = 256                   # [96:128, 256:268] (-2G)[:,23:35], partitions 96..127
BC_ONES = 268                 # [0:1, 268:396]    ones row (prefill lhsT)
BC_G1T2 = 396                 # [0:1, 396:780]    g1[23:35] tiled x32 (prefill rhs)
BW = 780

import os as _os
# T1-copy engine schedule (16 chunks of 512): 'd' DVE / 'a' ACT (deferred)
COPY_SCHED = _os.environ.get("K_COPYSCHED", "dddadddadddaddaa")
# square engine per 1024-group (8 groups): 'd' DVE TT / 'p' Pool TT
SQ_SCHED = _os.environ.get("K_SQSCHED", "dddddddd")

# ---- static exterior-derivative index maps (mirrors reference.py) ----
_IDX3 = list(combinations(range(DIM), 3))
_POS2 = {t: i for i, t in enumerate(combinations(range(DIM), 2))}
_D2 = []
for _out, (i, j, k) in enumerate(_IDX3):
    for _p, (a, b, c) in enumerate([(i, j, k), (j, i, k), (k, i, j)]):
        bc = tuple(sorted((b, c)))
        s = (-1) ** _p * (1 if (b, c) == bc else -1)
        _D2.append((_out, _POS2[bc], a, s))


def _build_G(W1: np.ndarray, W2: np.ndarray) -> np.ndarray:
    """G[j, o] = sum_t SIGNS[t] * sin(EPS*W1[DCOORD[t], j])/EPS * W2[j, IN_POS[t]]  (fp64)."""
    W1d = W1.astype(np.float64)
    W2d = W2.astype(np.float64)
    G = np.zeros((32, 35), dtype=np.float64)
    for out_pos, in_pos, dcoord, sign in _D2:
        G[:, out_pos] += sign * (np.sin(EPS * W1d[dcoord, :]) / EPS) * W2d[:, in_pos]
    return G


_PROG = None


def _get_prog():
    global _PROG
    if _PROG is not None:
        return _PROG
    import concourse.bacc as bacc
    import concourse.bass as bass
    import concourse.tile as tile
    import concourse.mybir as mybir
    from concourse.alu_op_type import AluOpType as Alu

    F32 = mybir.dt.float32
    F16 = mybir.dt.float16
    Sin = mybir.ActivationFunctionType.Sin
    Ident = mybir.ActivationFunctionType.Identity

    nc = bacc.Bacc("TRN2", target_bir_lowering=False, debug=False, num_devices=NCORES)
    xt = nc.dram_tensor("xt", [K_IN, COLS], F16, kind="ExternalInput")
    blob = nc.dram_tensor("blob", [128, BW], F16, kind="ExternalInput")
    g1f32 = nc.dram_tensor("g1f32", [128, 1], F32, kind="ExternalInput")
    ot = nc.dram_tensor("ot", [128, OUT_W], F16, kind="ExternalOutput")

    with tile.TileContext(nc) as tc:
        with (
            tc.tile_pool(name="singles", bufs=1) as singles,
            tc.tile_pool(name="xin", bufs=2) as xpool,
            tc.tile_pool(name="och", bufs=2) as opool,
            tc.tile_pool(name="o2ch", bufs=2) as o2pool,
            tc.tile_pool(name="ssp", bufs=2) as spool,
            tc.tile_pool(name="qqp", bufs=2) as qpool,
            tc.tile_pool(name="zps", bufs=2, space=bass.MemorySpace.PSUM) as zpsum,
            tc.tile_pool(name="t1ps", bufs=3, space=bass.MemorySpace.PSUM) as t1psum,
            tc.tile_pool(name="t2ps", bufs=1, space=bass.MemorySpace.PSUM) as t2psum,
        ):
            warm = singles.tile([1, 64], F16)
            nc.vector.memset(warm[:], 0.0)
            bl = singles.tile([128, BW], F16)
            nc.sync.dma_start(bl[0:K_IN, 0:128], blob[0:K_IN, 0:128])
            g1s = singles.tile([128, 1], F32)

            w1_ap = bl[0:K_IN, BC_W1:BC_W1 + 128]
            ga_ap = bl[0:128, BC_GA:BC_GA + 128]
            g2_ap = bl[96:128, BC_G2:BC_G2 + 12]
            ones_ap = bl[0:1, BC_ONES:BC_ONES + 128]
            g1t2_ap = bl[0:1, BC_G1T2:BC_G1T2 + T2_W]

            wps = t1psum.tile([128, 512], F32, tag="t1")
            nc.tensor.matmul(wps[0:1, 0:64], warm[0:1, 0:1], warm[0:1, 0:64])

            # group widths: short head groups (fast pipeline fill) and short
            # tail groups (short drain chain)
            widths = [512, 512, 1024, 1024, 1024, 1024, 1024, 1024, 512, 512]
            assert sum(widths) == COLS
            t2 = None
            tb = 0               # T2 block counter
            chunk = 0            # T1-copy chunk counter (16 total)
            pending_act = []     # deferred ACT copies (dst, t1) from prior group
            pending_store = []   # stores deferred until the prior group's
                                 # copies (incl. deferred ACT ones) are emitted
            xins = {}
            ochs = {}
            sss = {}
            qqs = {}
            c0 = 0
            ngrp = len(widths)
            for g, W in enumerate(widths):
                pi = c0 // PAIR
                po = c0 - pi * PAIR
                if pi not in xins:
                    xin = xpool.tile([K_IN, PAIR], F16, tag="xin")
                    if pi == 0:
                        # tiny first chunk via Pool SWDGE overlapping the
                        # w1 load on SP/HWDGE -> earliest possible mm1;
                        # weight-blob remainder + bias column follow the
                        # second x chunk so they don't delay sin(g1)
                        nc.gpsimd.dma_start(xin[:, :512], xt[:, :512])
                        nc.sync.dma_start(xin[:, 512:], xt[:, 512:PAIR])
                        nc.sync.dma_start(bl[:, 128:], blob[:, 128:])
                        nc.sync.dma_start(g1s[:], g1f32[:])
                    else:
                        nc.sync.dma_start(xin[:], xt[:, pi * PAIR:(pi + 1) * PAIR])
                    xins[pi] = xin
                    och = opool.tile([128, PAIR], F16, tag="och")
                    ss = spool.tile([128, PAIR], F16, tag="ss")
                    qq = qpool.tile([128, PAIR], F16, tag="qq")
                    ochs[pi], sss[pi], qqs[pi] = och, ss, qq
                xin, och, ss, qq = xins[pi], ochs[pi], sss[pi], qqs[pi]
                last = g == ngrp - 1

                zp = zpsum.tile([128, GROUP], F32, tag="zp")
                for s in range(0, W, 512):
                    nc.tensor.matmul(zp[:, s:s + 512], w1_ap,
                                     xin[:, po + s:po + s + 512])
                nc.scalar.activation(ss[:, po:po + W], zp[:, :W], Sin,
                                     bias=0.0, scale=0.5)
                # deferred ACT copies sit after this sin in the ACT queue, so
                # their mm2A dependency is long satisfied -> no ACT stall
                for dst, t1p in pending_act:
                    nc.scalar.activation(dst, t1p[:], Ident,
                                         bias=g1s[:], scale=1.0)
                pending_act = []
                for eng, *dma_args in pending_store:
                    eng.dma_start(*dma_args)
                pending_store = []
                nc.vector.tensor_tensor(qq[:, po:po + W], ss[:, po:po + W],
                                        ss[:, po:po + W], Alu.mult)

                # leftover 12 outputs of subgroup 3 (q-stationary matmuls);
                # emitted before mm2A in the last group so the final T2 copy
                # (on ACT) runs parallel to the final T1 copies (on DVE)
                def t2_blocks():
                    nonlocal t2, tb
                    for b in range(W // T2_BLK):
                        w = tb % T2_PER_TILE
                        if w == 0:
                            t2 = t2psum.tile([128, T2_W], F32, tag="t2")
                            nc.tensor.matmul(t2[:], ones_ap, g1t2_ap,
                                             start=True, stop=False)
                        qb = qq[96:128, po + b * T2_BLK:po + (b + 1) * T2_BLK]
                        nc.tensor.matmul(t2[:, 12 * w:12 * w + 12], qb, g2_ap,
                                         start=False, stop=(w == T2_PER_TILE - 1),
                                         tile_position=(96, 0))
                        if w == T2_PER_TILE - 1:
                            o2 = o2pool.tile([128, T2_W], F16, tag="o2")
                            t2o = COLS + (tb // T2_PER_TILE) * T2_W
                            nc.vector.tensor_copy(o2[:], t2[:])
                            (nc.scalar if tb == NT2 - 1 else nc.gpsimd
                             ).dma_start(ot[:, t2o:t2o + T2_W], o2[:])
                        tb += 1

                # main output stream: 3 samples + 23/35 of a 4th per column
                for s in range(0, W, 512):
                    t1 = t1psum.tile([128, 512], F32, tag="t1")
                    nc.tensor.matmul(t1[:], ga_ap, qq[:, po + s:po + s + 512])
                    dst = och[:, po + s:po + s + 512]
                    e = COPY_SCHED[chunk % len(COPY_SCHED)]
                    chunk += 1
                    if e == 'a' and not last:
                        pending_act.append((dst, t1))
                    elif e == 'a':
                        nc.scalar.activation(dst, t1[:], Ident,
                                             bias=g1s[:], scale=1.0)
                    else:
                        nc.vector.tensor_scalar(dst, t1[:], g1s[:], None,
                                                Alu.add)

                if not last:
                    if g == ngrp - 2:
                        pass       # merged into the final store below
                    else:
                        seng = nc.gpsimd if g < 6 else nc.sync
                        pending_store.append(
                            (seng, ot[:, c0:c0 + W], och[:, po:po + W]))
                    t2_blocks()
                else:
                    for dst, t1p in pending_act:
                        nc.scalar.activation(dst, t1p[:], Ident,
                                             bias=g1s[:], scale=1.0)
                    pending_act = []
                    for eng, *dma_args in pending_store:
                        eng.dma_start(*dma_args)
                    pending_store = []
                    t2_blocks()
                    nc.sync.dma_start(ot[:, c0 - widths[g - 1]:c0 + W],
                                      och[:, po - widths[g - 1]:po + W])
                c0 += W

    nc.compile()
    _PROG = nc
    return nc


def _pack_inputs(x: np.ndarray, W1: np.ndarray, W2: np.ndarray):
    assert x.shape == (B, DIM), x.shape
    assert W1.shape == (DIM, 32), W1.shape
    assert W2.shape == (32, 21), W2.shape
    G = _build_G(W1, W2)                      # fp64 (32, 35)
    Gm2 = (-2.0 * G).astype(np.float16)       # (32, 35)
    g1 = G.sum(axis=0)                        # (35,)

    g1s128 = np.empty(128, dtype=np.float64)
    for h in range(3):
        g1s128[35 * h:35 * h + 35] = g1
    g1s128[105:128] = g1[:23]

    blob = np.zeros((128, BW), dtype=np.float16)
    for gsub in range(SUB):
        blob[7 * gsub:7 * gsub + 7, BC_W1 + 32 * gsub:BC_W1 + 32 * gsub + 32] = \
            W1.astype(np.float16)
    for h in range(3):
        blob[32 * h:32 * h + 32, BC_GA + 35 * h:BC_GA + 35 * h + 35] = Gm2
    blob[96:128, BC_GA + 105:BC_GA + 128] = Gm2[:, :23]
    blob[96:128, BC_G2:BC_G2 + 12] = Gm2[:, 23:35]
    blob[0, BC_ONES:BC_ONES + 128] = 1.0
    blob[0, BC_G1T2:BC_G1T2 + T2_W] = np.tile(g1[23:35], T2_PER_TILE).astype(np.float16)

    # xt[m][7g+f, c] = x[m*B_CORE + g*COLS + c, f]
    xr = np.asarray(x, dtype=np.float16).reshape(NCORES, SUB, COLS, DIM)
    xt = np.ascontiguousarray(xr.transpose(0, 1, 3, 2).reshape(NCORES, K_IN, COLS))
    g1f = np.ascontiguousarray(g1s128[:, None], dtype=np.float32)
    in_maps = [{"xt": xt[m], "blob": blob, "g1f32": g1f} for m in range(NCORES)]
    return in_maps


def _unpack_outputs(results) -> np.ndarray:
    ot = np.stack([r["ot"] for r in results])       # (8, 128, 8960) f16
    A = ot[:, :, :COLS]                             # (8, 128, 8192)
    # subgroups 0..2: rows 35h+o
    a3 = A[:, :105, :].reshape(NCORES, 3, 35, COLS).transpose(0, 1, 3, 2)
    # subgroup 3 outputs 0..22: rows 105..127
    a4 = A[:, 105:128, :].transpose(0, 2, 1)        # (8, 8192, 23)
    # subgroup 3 outputs 23..34: T2 region [128 samples, 12] blocks
    Bp = ot[:, :, COLS:].reshape(NCORES, 128, NT2 // T2_PER_TILE, T2_PER_TILE, 12)
    b4 = Bp.transpose(0, 2, 3, 1, 4).reshape(NCORES, COLS, 12)
    out4 = np.concatenate([a4, b4], axis=2)         # (8, 8192, 35)
    out = np.concatenate([a3.reshape(NCORES, 3 * COLS, 35), out4], axis=1)
    return np.ascontiguousarray(out.reshape(B, 35), dtype=np.float32)


def run(x, W1, W2, **spmd_kwargs):
    """Run the kernel; returns (output, BassKernelResults)."""
    from concourse.bass_utils import run_bass_kernel_spmd
    nc = _get_prog()
    in_maps = _pack_inputs(np.asarray(x, dtype=np.float32),
                           np.asarray(W1, dtype=np.float32),
                           np.asarray(W2, dtype=np.float32))
    res = run_bass_kernel_spmd(nc, in_maps, core_ids=list(range(NCORES)), **spmd_kwargs)
    return _unpack_outputs(res.results), res


def kernel(x, W1, W2):
    out, _ = run(x, W1, W2)
    return out
